# revision 16
# baseline (speedup 1.0000x reference)
"""Trainium2 Bass kernel for nn_CubicExpansion (cubic spline basis expansion).

reference semantics (k=32 knots, uniform linspace grid):
    j   = clip(searchsorted(xk, x), 1, k-1);  b = j-1
    h   = xk[j]-xk[j-1] (== step, uniform grid)
    u   = (x - xk[b]) / step
    out = c_jm*F[b] + c_jp*F[b+1] + a_jm*e_b + a_jp*e_{b+1}
with a_jm=1-u, a_jp=u, c_jm=s2*((1-u)^3-(1-u)), c_jp=s2*(u^3-u), s2=step^2/6.

Key identity used here: with z=(x-x0)/step and d_m = z-m,
    a-part weight on column m  ==  r_m           where r_m = relu(1-|d_m|)
    F-row weight on row m      ==  s2*(r_m^3-r_m)
(both symmetric in d_m, compactly supported on |d_m|<1), so the entire row is
    out[i,:] = W_F[i,:] @ F + r[i,:]
with W_F, r built from x by a handful of wide elementwise ops -- no
searchsorted, no gather, no one-hot compares.

Per 128-point column tile: PE transposes W_F ([128,32] -> [32,128], four tiles
per 128-wide transpose), then K=32 row-group-packed matmuls against the
constant F table produce out_F[128,32] directly in PSUM; DVE adds r during
PSUM evacuation. Pure data parallel across the 8 NeuronCores.
"""

import os

import numpy as np

LAST_EXEC_NS = None

N_POINTS = 4_000_000
K = 32
NCORES = 8
P = 128
FP = 3968            # point-columns per core (128*3968 = 507904 pts/core)
FC = 64              # point-columns per group
NG = FP // FC        # 62 groups
PTS_PER_CORE = P * FP
NPAD = NCORES * PTS_PER_CORE  # 4063232


def _get_F_np(xk64):
    """F matrix from Wood (2017), faithful numpy port of reference _get_F."""
    k = xk64.shape[0]
    h = np.diff(xk64)
    hs = h[1:]
    hm = h[: k - 2]
    i = np.arange(k - 2)
    D = np.zeros((k - 2, k), dtype=xk64.dtype)
    D[i, i] = 1.0 / hm
    D[i, i + 1] = -1.0 / hm - 1.0 / hs
    D[i, i + 2] = 1.0 / hs
    B = np.zeros((k - 2, k - 2), dtype=xk64.dtype)
    B[i, i] = (hm + hs) / 3.0
    off = hs[k - 3] / 6.0
    ii = np.arange(k - 3)
    B[ii, ii + 1] = off
    B[ii + 1, ii] = off
    F_minus = np.linalg.solve(B, D)
    zrow = np.zeros((1, k), dtype=xk64.dtype)
    return np.vstack([zrow, F_minus, zrow])


def _build_nc(istep, zbias, s2, stage=3):
    """Trace the Bass program. istep/zbias/s2 are baked fp32 immediates."""
    from contextlib import ExitStack

    import concourse.bass as bass  # noqa: F401
    import concourse.tile as tile
    from concourse import bacc, mybir

    istep, zbias, s2 = float(istep), float(zbias), float(s2)
    f32 = mybir.dt.float32
    nc = bacc.Bacc("TRN2")

    x_d = nc.dram_tensor("x", [P, FP], f32, kind="ExternalInput")
    iota_d = nc.dram_tensor("iota", [P, FC * K], f32, kind="ExternalInput")
    ident_d = nc.dram_tensor("ident", [P, P], f32, kind="ExternalInput")
    fmat_d = nc.dram_tensor("fmat", [P, P], f32, kind="ExternalInput")
    out_d = nc.dram_tensor("out", [P, FP * K], f32, kind="ExternalOutput")
    out_v = out_d[:, :].rearrange("p (f m) -> p f m", m=K)

    with ExitStack() as ctx:
        tc = ctx.enter_context(tile.TileContext(nc))
        singles = ctx.enter_context(tc.tile_pool(name="singles", bufs=1))
        xz = ctx.enter_context(tc.tile_pool(name="xz", bufs=2))
        chain = ctx.enter_context(tc.tile_pool(name="chain", bufs=2))
        wtp = ctx.enter_context(tc.tile_pool(name="wtp", bufs=3))
        outp = ctx.enter_context(tc.tile_pool(name="outp", bufs=2))
        psw = ctx.enter_context(tc.tile_pool(name="psw", bufs=2, space="PSUM"))
        pso = ctx.enter_context(tc.tile_pool(name="pso", bufs=2, space="PSUM"))

        # constants, loaded once
        x_all = singles.tile([P, FP], f32)
        nc.sync.dma_start(out=x_all[:, :], in_=x_d[:, :])
        iota_t = singles.tile([P, FC, K], f32)
        nc.sync.dma_start(out=iota_t[:, :, :], in_=iota_d[:, :].rearrange("p (f m) -> p f m", m=K))
        ident = singles.tile([P, P], f32)
        nc.sync.dma_start(out=ident[:, :], in_=ident_d[:, :])
        fmat = singles.tile([P, P], f32)
        nc.sync.dma_start(out=fmat[:, :], in_=fmat_d[:, :])

        for g in range(NG):
            # z = (x - x0)/step  over this group's 64 point-columns
            z = xz.tile([P, FC], f32)
            nc.vector.tensor_scalar(
                out=z[:, :], in0=x_all[:, g * FC:(g + 1) * FC],
                scalar1=istep, scalar2=zbias,
                op0=mybir.AluOpType.mult, op1=mybir.AluOpType.add,
            )
            z_b = z[:, :].unsqueeze(2).to_broadcast([P, FC, K])

            # d = iota - z  (sign irrelevant downstream)
            dd = chain.tile([P, FC, K], f32, tag="dd")
            nc.vector.tensor_tensor(out=dd[:, :, :], in0=iota_t[:, :, :], in1=z_b,
                                    op=mybir.AluOpType.subtract)
            # ad = |d|
            ad = chain.tile([P, FC, K], f32, tag="ad")
            nc.scalar.activation(ad[:, :, :], dd[:, :, :], mybir.ActivationFunctionType.Abs)
            # r = relu(1 - |d|)   == a-part row
            r = chain.tile([P, FC, K], f32, tag="r")
            nc.scalar.activation(r[:, :, :], ad[:, :, :], mybir.ActivationFunctionType.Relu,
                                 bias=1.0, scale=-1.0)
            # sq = r^2
            sq = chain.tile([P, FC, K], f32, tag="sq")
            nc.scalar.activation(sq[:, :, :], r[:, :, :], mybir.ActivationFunctionType.Square)
            # inner = s2*sq - s2
            inner = chain.tile([P, FC, K], f32, tag="inner")
            nc.vector.tensor_scalar(
                out=inner[:, :, :], in0=sq[:, :, :], scalar1=s2, scalar2=s2,
                op0=mybir.AluOpType.mult, op1=mybir.AluOpType.subtract,
            )
            # WF = r * inner == s2*(r^3 - r)
            wf = chain.tile([P, FC, K], f32, tag="wf")
            nc.vector.tensor_tensor(out=wf[:, :, :], in0=r[:, :, :], in1=inner[:, :, :],
                                    op=mybir.AluOpType.mult)

            if stage == 0:
                # debug: write wf + r directly (no PE)
                for s in range(FC // 16):
                    ob = outp.tile([P, 16, K], f32, tag="ob")
                    nc.vector.tensor_tensor(
                        out=ob[:, :, :],
                        in0=wf[:, s * 16:(s + 1) * 16, :],
                        in1=r[:, s * 16:(s + 1) * 16, :],
                        op=mybir.AluOpType.add,
                    )
                    nc.sync.dma_start(
                        out=out_v[:, g * FC + s * 16: g * FC + (s + 1) * 16, :],
                        in_=ob[:, :, :],
                    )
                continue

            # 16 point-columns per psum mega-tile
            for s in range(FC // 16):
                ps_w = psw.tile([P, 512], f32, tag="ps_w")
                for t in range(4):
                    c0 = s * 16 + t * 4  # first of 4 point-columns
                    nc.tensor.transpose(
                        out=ps_w[:, t * 128:(t + 1) * 128],
                        in_=wf[:, c0:c0 + 4, :],
                        identity=ident[:, :],
                    )
                wt = wtp.tile([P, 512], f32, tag="wt")
                nc.scalar.activation(wt[:, :], ps_w[:, :], mybir.ActivationFunctionType.Copy)

                if stage == 1:
                    # debug: write wt back (transpose path only, layout scrambled)
                    ob = outp.tile([P, 16, K], f32, tag="ob")
                    nc.vector.tensor_copy(ob[:, :, :], wt[:, :].rearrange("p (f m) -> p f m", m=K))
                    nc.sync.dma_start(
                        out=out_v[:, g * FC + s * 16: g * FC + (s + 1) * 16, :],
                        in_=ob[:, :, :],
                    )
                    continue

                # one K=128 matmul per 4 point-columns: lhsT carries 4 stacked
                # W^T tiles; block-diagonal rhs keeps them in separate 32-col
                # output blocks.
                ps_o = pso.tile([P, 512], f32, tag="ps_o")
                for t in range(4):
                    nc.tensor.matmul(
                        out=ps_o[:, t * 128:(t + 1) * 128],
                        lhsT=wt[:, t * 128:(t + 1) * 128],
                        rhs=fmat[:, :],
                        start=True, stop=True,
                    )
                # out = out_F + r   (a-part added during PSUM evacuation)
                ob = outp.tile([P, 16, K], f32, tag="ob")
                nc.vector.tensor_tensor(
                    out=ob[:, :, :],
                    in0=ps_o[:, :].rearrange("p (f m) -> p f m", m=K),
                    in1=r[:, s * 16:(s + 1) * 16, :],
                    op=mybir.AluOpType.add,
                )
                nc.sync.dma_start(
                    out=out_v[:, g * FC + s * 16: g * FC + (s + 1) * 16, :],
                    in_=ob[:, :, :],
                )
    if not nc.is_finalized():
        nc.finalize()
    return nc


def _install_ntff_shim():
    """Provide antenv.axon_hooks (missing in this image) so bass_utils can
    NTFF-profile through the axon PJRT .so. Best-effort."""
    import contextlib
    import ctypes
    import sys
    import types

    if "antenv.axon_hooks" in sys.modules:
        return
    so_path = "/opt/axon/libaxon_pjrt.so"
    if not os.path.exists(so_path):
        return
    lib = ctypes.CDLL(so_path)
    if not hasattr(lib, "axon_start_nrt_profile"):
        return
    lib.axon_start_nrt_profile.argtypes = [
        ctypes.POINTER(ctypes.c_int64),
        ctypes.c_size_t,
    ]
    lib.axon_start_nrt_profile.restype = ctypes.c_int64
    lib.axon_stop_nrt_profile.argtypes = [ctypes.c_char_p]
    lib.axon_stop_nrt_profile.restype = ctypes.c_int64

    @contextlib.contextmanager
    def _hook(output_dir, device_ids):
        import jax

        jax.devices()
        if device_ids:
            ids = (ctypes.c_int64 * len(device_ids))(*device_ids)
            rc = lib.axon_start_nrt_profile(ids, len(device_ids))
        else:
            rc = lib.axon_start_nrt_profile(None, 0)
        if rc != 0:
            raise RuntimeError(f"axon_start_nrt_profile rc={rc}")
        try:
            yield
        finally:
            n = lib.axon_stop_nrt_profile(str(output_dir).encode())
            print(f"ntff profile: {n} file(s) written to {output_dir}")

    holder = [_hook]
    mod = types.ModuleType("antenv.axon_hooks")
    mod.get_axon_ntff_profile_hook = lambda: holder[0]
    mod.set_axon_ntff_profile_hook = lambda h: holder.__setitem__(0, h)
    sys.modules["antenv.axon_hooks"] = mod
    try:
        import antenv

        antenv.axon_hooks = mod
    except ImportError:
        pass


def kernel(x, xk):
    x = np.asarray(x, dtype=np.float32)
    xk = np.asarray(xk, dtype=np.float32)

    # host-side tiny precompute (O(k^2))
    F = _get_F_np(xk.astype(np.float64)).astype(np.float32)
    x0 = float(xk[0])
    step = (float(xk[K - 1]) - x0) / (K - 1)
    istep = 1.0 / step
    zbias = -x0 * istep
    s2 = step * step / 6.0

    nc = _build_nc(np.float32(istep), np.float32(zbias), np.float32(s2))

    # shard + pad x:  core c owns points [c*PTS_PER_CORE, (c+1)*PTS_PER_CORE)
    xp = np.full(NPAD, x0, dtype=np.float32)
    xp[:N_POINTS] = x
    xs = xp.reshape(NCORES, P, FP)

    iota = np.tile(np.arange(K, dtype=np.float32), (P, FC)).reshape(P, FC * K)
    ident = np.eye(P, dtype=np.float32)
    fmat = np.kron(np.eye(4, dtype=np.float32), F)  # [128, 128] block-diag

    from concourse.bass_utils import run_bass_kernel_spmd

    in_maps = [
        {"x": xs[c], "iota": iota, "ident": ident, "fmat": fmat}
        for c in range(NCORES)
    ]
    trace = os.environ.get("KERNEL_TRACE", "0") == "1"
    if trace:
        try:
            _install_ntff_shim()
        except Exception as e:  # noqa: BLE001
            print("ntff shim failed:", e)
            trace = False
    try:
        res = run_bass_kernel_spmd(
            nc, in_maps, core_ids=list(range(NCORES)), trace=trace,
            trace_cores=[0] if trace else None,
        )
    except Exception:
        if not trace:
            raise
        res = run_bass_kernel_spmd(nc, in_maps, core_ids=list(range(NCORES)))
    global LAST_EXEC_NS
    LAST_EXEC_NS = res.exec_time_ns

    out = np.empty((NPAD, K), dtype=np.float32)
    for c in range(NCORES):
        out[c * PTS_PER_CORE:(c + 1) * PTS_PER_CORE] = (
            res.results[c]["out"].reshape(PTS_PER_CORE, K)
        )
    return out[:N_POINTS]


if __name__ == "__main__":
    rng = np.random.default_rng(0)
    x = rng.random(N_POINTS, dtype=np.float32)
    xk = np.linspace(x.min(), x.max(), K).astype(np.float32)
    out = kernel(x, xk)
    print(out.shape, out.dtype)
